# revision 28
# baseline (speedup 1.0000x reference)
"""YOLO-style loss kernel for Trainium2, 8-core data parallel.

Per cell (B=1, C=21; conf target is exactly 0.0 or 1.0):
  sig(v)    = exp(-ln(1+exp(-v)))                       (Exp/Ln only)
  loss_xy   = sum(conf * (sig(p0) - x_t)^2)
  loss_wh   = sum(conf * (p1 - ln(w_t/SCALE))^2)
  dc        = sig(p2) - conf ; e_tot = sum(dc^2) ; obj = sum(conf*dc^2)
  s   = sum_c exp(l_c) ; NN10 = 10*sum_c exp(l_c)*(1+0.5c)
  diff = 10*(pm+1)/(ct+1) - 10 = (NN10 - 10*s*ct)/(s*(ct+1))
  adiff = |diff| ; mm = min(adiff, conf*min(ct,1))
  loss_cls += sum(mm * (adiff - 0.5*mm))                (huber, masked)

The per-cell class sums s and NN10 are computed on the otherwise-idle
TensorEngine: exp(logits) [128, K*21] is transposed in [128,126] chunks
(126 = 6 cells x 21 classes) into PSUM, bounced to SBUF, then each chunk
is a matmul against a constant [126,12] weight (columns = per-cell ones
and 10*mass) accumulating into a [128, 2K] PSUM bank laid out so column
2k+t is exactly (s, NN10) for cell k.  ACT does exp/ln/square-accum,
Pool (gpsimd) the small elementwise chain, DVE only PSUM evacuation and
the custom-op reductions.  One explicit LoadActFuncSet of the combined
exp+ln table avoids table thrashing.
Per-core output: [128, 8] partial sums (xy, wh, obj, e_tot, cls, 0..).
"""

import numpy as np

S = 10
NCOMP = 24
NCLS = 21
SCALE = 6.5131 / 40.0
BATCH = 32768
NCORES = 8
CB = BATCH // NCORES            # 4096 rows per core
CELLS = CB * S * S              # 409600 cells per core
P = 128
CPP = CELLS // P                # 3200 cells per partition
K = 200                         # cells per partition per tile
KTAIL = 50                      # last big tile is split 4x for short drain
NT = CPP // K                   # 16 tiles
CHUNK = 126                     # 6 cells x 21 classes per transpose chunk
GRP = 8                         # transpose chunks per PSUM staging bank
# segments: (kk, tile-index in the kk-view, slot column)
SEGS = [(K, t) for t in range(NT)]
KVIEWS = (K,)
NSLOT = len(SEGS)

_CACHE = {}


def _wconst_np():
    w = np.zeros((P, 12), dtype=np.float32)
    for kl in range(6):
        for c in range(NCLS):
            w[kl * NCLS + c, 2 * kl] = 1.0
            w[kl * NCLS + c, 2 * kl + 1] = 10.0 + 5.0 * c
    return w


def _build_nc():
    import concourse.bacc as bacc
    import concourse.tile as tile
    import concourse.mybir as mybir
    from concourse.hw_specs import get_activation_tables
    from concourse.masks import make_identity

    f32 = mybir.dt.float32
    bf16 = mybir.dt.bfloat16
    AF = mybir.ActivationFunctionType
    OP = mybir.AluOpType
    AX = mybir.AxisListType

    nc = bacc.Bacc("TRN2", target_bir_lowering=False, debug=False)
    pred = nc.dram_tensor("pred", [CB, S, S, NCOMP], f32, kind="ExternalInput").ap()
    tgt = nc.dram_tensor("tgt", [CB, S, S, 4], f32, kind="ExternalInput").ap()
    wdram = nc.dram_tensor("wconst", [P, 12], f32, kind="ExternalInput").ap()
    out = nc.dram_tensor("out", [P, 8], f32, kind="ExternalOutput").ap()

    pf = pred.flatten_outer_dims()
    gf = tgt.flatten_outer_dims()
    views = {}
    for kk in KVIEWS:
        views[kk] = (
            pf.rearrange("(t p k) c -> t p k c", p=P, k=kk),
            gf.rearrange("(t p k) c -> t p k c", p=P, k=kk),
        )

    tabs = list(get_activation_tables(nc.m.arch).items())
    combined = None
    for i, (name, funcs) in enumerate(tabs):
        fs = {str(f).split(".")[-1] for f in funcs}
        if {"Exp", "Ln", "Square"} <= fs:
            combined = i
            break
    assert combined is not None

    with tile.TileContext(nc) as tc:
        with (
            tc.tile_pool(name="singles", bufs=1) as singles,
            tc.tile_pool(name="io", bufs=4) as io,
            tc.tile_pool(name="big", bufs=3) as big,
            tc.tile_pool(name="elt", bufs=3) as eltp,
            tc.tile_pool(name="small", bufs=2) as small,
            tc.tile_pool(name="carry", bufs=3) as carry,
            tc.tile_pool(name="stg", bufs=4, space="PSUM") as stgp,
            tc.tile_pool(name="pout", bufs=3, space="PSUM") as poutp,
        ):
            def stile(pool, tag, kk, inner=None, dt=f32):
                shape = [P, K] if inner is None else [P, K, inner]
                t_ = pool.tile(shape, dt, tag=tag)
                return t_[:, 0:kk] if inner is None else t_[:, 0:kk, :]

            nc.scalar.add_instruction(mybir.InstLoadActFuncSet(
                name=nc.get_next_instruction_name(),
                act_func_set_id=combined, ins=[], outs=[]))

            ident = singles.tile([P, P], bf16)
            make_identity(nc, ident)
            wtile32 = singles.tile([P, 12], f32)
            nc.sync.dma_start(out=wtile32, in_=wdram)
            wtile = singles.tile([P, 12], bf16)
            nc.vector.tensor_copy(wtile, wtile32)
            slots = singles.tile([P, 5, NSLOT], f32)
            outacc = singles.tile([P, 8], f32)
            nc.vector.memset(outacc, 0.0)

            def head(kk, t):
                """DMA, exponentials, PE class sums, early Pool ops."""
                p_t, g_t = views[kk]
                sfx = "" if kk == K else f"_{kk}"
                pt = stile(io, "pt", kk, NCOMP)
                gt = stile(io, "gt", kk, 4)
                nc.sync.dma_start(out=pt, in_=p_t[t])
                nc.sync.dma_start(out=gt, in_=g_t[t])

                x_conf = gt[:, :, 0:4].rearrange(
                    "p k (a b) -> p k a b", a=2)[:, :, :, 0]
                conf = gt[:, :, 2]
                ct = gt[:, :, 3]
                xc = pt[:, :, 0:4].rearrange(
                    "p k (a b) -> p k a b", a=2)[:, :, :, 0]

                el = stile(big, "el", kk, NCLS, bf16)
                nc.scalar.activation(el, pt[:, :, 3:24], AF.Exp)
                ex2 = stile(small, "ex2", kk, 2)
                nc.scalar.activation(ex2, xc, AF.Exp, scale=-1.0)
                ln12 = stile(small, "ln12", kk, 2)
                nc.scalar.activation(ln12, ex2, AF.Ln, bias=1.0)
                sig2 = stile(small, "sig2", kk, 2)
                nc.scalar.activation(sig2, ln12, AF.Exp, scale=-1.0)
                lnw = stile(small, "lnw", kk)
                nc.scalar.activation(lnw, gt[:, :, 1], AF.Ln, scale=1.0 / SCALE)

                return dict(pt=pt, gt=gt, el=el, sig2=sig2, lnw=lnw,
                            x_conf=x_conf, conf=conf, ct=ct, kk=kk, sfx=sfx)

            def head_pe(h):
                """PE transposes + evacs + matmuls + snn + Pool block."""
                kk = h["kk"]
                sfx = h["sfx"]
                pt = h["pt"]
                el = h["el"]
                sig2 = h["sig2"]
                lnw = h["lnw"]
                x_conf = h["x_conf"]
                conf = h["conf"]
                ct = h["ct"]
                # PE: transpose el chunks, matmul with W -> (s, NN10)
                elf = el.rearrange("p k c -> p (k c)")
                po = poutp.tile([P, 2 * K], f32, tag="po")
                nch = (kk * NCLS + CHUNK - 1) // CHUNK
                ng = (nch + GRP - 1) // GRP
                for g in range(ng):
                    j0 = g * GRP
                    jn = min(GRP, nch - j0)
                    stg = stgp.tile([CHUNK, GRP * P], bf16, tag="stg")
                    et = eltp.tile([CHUNK, GRP * P], bf16, tag="et")
                    for i in range(jn):
                        j = j0 + i
                        c0 = j * CHUNK
                        w = min(CHUNK, kk * NCLS - c0)
                        nc.tensor.transpose(
                            stg[0:w, i * P:(i + 1) * P], elf[:, c0:c0 + w], ident)
                    wlast = min(CHUNK, kk * NCLS - (j0 + jn - 1) * CHUNK)
                    if wlast == CHUNK:
                        nc.vector.tensor_copy(et[:, 0:jn * P], stg[:, 0:jn * P])
                    else:
                        if jn > 1:
                            nc.vector.tensor_copy(et[:, 0:(jn - 1) * P],
                                                  stg[:, 0:(jn - 1) * P])
                        nc.vector.tensor_copy(
                            et[0:wlast, (jn - 1) * P:jn * P],
                            stg[0:wlast, (jn - 1) * P:jn * P])
                    for i in range(jn):
                        j = j0 + i
                        w = min(CHUNK, kk * NCLS - j * CHUNK)
                        ncol = (w // NCLS) * 2
                        nc.tensor.matmul(
                            po[:, 12 * j:12 * j + ncol],
                            et[0:w, i * P:(i + 1) * P],
                            wtile[0:w, 0:ncol],
                            start=True, stop=True)

                snn = stile(carry, "snn", kk, 2)
                nc.vector.tensor_copy(snn.rearrange("p k c -> p (k c)"), po[:, 0:2 * kk])

                # Pool: early elementwise (no class-sum dependency)
                dxdc = stile(carry, "dxdc", kk, 2)
                nc.gpsimd.tensor_sub(dxdc, sig2, x_conf)
                dw = stile(small, "dw", kk)
                nc.gpsimd.tensor_sub(dw, pt[:, :, 1], lnw)
                vx = stile(carry, "vx", kk)
                nc.gpsimd.tensor_mul(vx, dxdc[:, :, 0], conf)
                vw = stile(carry, "vw", kk)
                nc.gpsimd.tensor_mul(vw, dw, conf)
                g_ = stile(small, "g_", kk)
                nc.gpsimd.tensor_scalar_min(g_, ct, 1.0)
                cmask = stile(carry, "cmask", kk)
                nc.gpsimd.tensor_mul(cmask, g_, conf)
                h.update(snn=snn, dxdc=dxdc, vx=vx, vw=vw, cmask=cmask)
                return h

            def tail(t, h):
                """Class-loss chain + final reductions for tile t (delayed
                one iteration so queued ops never head-of-line block)."""
                kk = h["kk"]
                sfx = h["sfx"]
                conf = h["gt"][:, :, 2]
                ct = h["gt"][:, :, 3]
                s_ = h["snn"][:, :, 0]
                nn10 = h["snn"][:, :, 1]
                dc = h["dxdc"][:, :, 1]

                q = stile(small, "q", kk)
                nc.vector.tensor_mul(q, s_, ct)
                sp = stile(small, "sp", kk)
                nc.vector.tensor_add(sp, s_, q)
                rsp = stile(small, "rsp", kk)
                nc.vector.reciprocal_approx_fast(rsp, sp)
                numer = stile(small, "numer", kk)
                nc.vector.scalar_tensor_tensor(
                    numer, q, -10.0, nn10, op0=OP.mult, op1=OP.add
                )
                d10 = stile(small, "d10", kk)
                nc.vector.tensor_mul(d10, numer, rsp)
                absd = stile(small, "absd", kk)
                nc.vector.scalar_tensor_tensor(
                    absd, d10, -1.0, d10, op0=OP.mult, op1=OP.max
                )
                mm = stile(small, "mm", kk)
                nc.vector.tensor_tensor(mm, absd, h["cmask"], op=OP.min)
                st = stile(small, "st", kk)
                nc.vector.scalar_tensor_tensor(
                    st, mm, -0.5, absd, op0=OP.mult, op1=OP.add
                )

                scr = stile(small, "scr", kk)
                nc.scalar.activation(scr, h["vx"], AF.Square,
                                     accum_out=slots[:, 0, t:t + 1])
                scr2 = stile(small, "scr2", kk)
                nc.scalar.activation(scr2, h["vw"], AF.Square,
                                     accum_out=slots[:, 1, t:t + 1])
                dc2 = stile(small, "dc2", kk)
                nc.scalar.activation(dc2, dc, AF.Square,
                                     accum_out=slots[:, 3, t:t + 1])
                scr3 = stile(small, "scr3", kk)
                nc.vector.tensor_mul(scr3, dc2, conf)
                nc.vector.reduce_sum(slots[:, 2, t:t + 1], scr3, axis=AX.X)
                scr4 = stile(small, "scr4", kk)
                nc.vector.tensor_mul(scr4, mm, st)
                nc.vector.reduce_sum(slots[:, 4, t:t + 1], scr4, axis=AX.X)

            prev = None
            ntap = 3            # final taper segments: PE-first emission
            for sidx, (kk, t) in enumerate(SEGS):
                h = head(kk, t)
                if sidx < len(SEGS) - ntap:
                    if prev is not None:
                        tail(sidx - 1, prev)
                    head_pe(h)
                else:
                    head_pe(h)
                    if prev is not None:
                        tail(sidx - 1, prev)
                prev = h
            tail(len(SEGS) - 1, prev)

            for i in range(5):
                nc.vector.reduce_sum(outacc[:, i:i + 1], slots[:, i, :], axis=AX.X)
            nc.sync.dma_start(out=out, in_=outacc)
    nc.finalize()
    return nc


def _get_nc():
    if "nc" not in _CACHE:
        _CACHE["nc"] = _build_nc()
    return _CACHE["nc"]


def run_sharded(pred_tensor, target_tensor, trace=False):
    from concourse.bass_utils import run_bass_kernel_spmd

    nc = _get_nc()
    pred_np = np.ascontiguousarray(np.asarray(pred_tensor, dtype=np.float32))
    tgt_np = np.ascontiguousarray(np.asarray(target_tensor, dtype=np.float32))
    wc = _wconst_np()
    in_maps = [
        {
            "pred": pred_np[i * CB : (i + 1) * CB],
            "tgt": tgt_np[i * CB : (i + 1) * CB],
            "wconst": wc,
        }
        for i in range(NCORES)
    ]
    res = run_bass_kernel_spmd(nc, in_maps, core_ids=list(range(NCORES)), trace=trace)
    return [r["out"] for r in res.results], res


def kernel(pred_tensor, target_tensor):
    partials, _ = run_sharded(pred_tensor, target_tensor, trace=False)
    tot = np.zeros(8, dtype=np.float64)
    for p in partials:
        tot += p.astype(np.float64).sum(axis=0)
    xy, wh, obj, e_tot, cls = tot[0], tot[1], tot[2], tot[3], tot[4]
    noobj = e_tot - obj
    loss = 10.0 * (xy + wh) + obj + 1.0 * noobj + 0.5 * cls
    inv = 1.0 / BATCH
    return np.array(
        [xy * inv, wh * inv, obj * inv, noobj * inv, cls * inv, loss * inv],
        dtype=np.float32,
    )
